# revision 31
# baseline (speedup 1.0000x reference)
"""Trainium2 Bass kernel for nn_DispersiveLoss (B=2048, D=16*768=12288, 8 cores).

Single-launch circulant scheme, no cross-core communication:
  x (2048, 12288) -> 16 row-blocks of 128. Core c owns m-blocks {2c, 2c+1} and
  computes two raw Gram strips G[m, m+1..m+8 (mod 16)] ([128,1024], fp8
  DoubleRow, D on partitions, fp32 PSUM) plus the two self blocks G[m,m] in a
  separate [128,256] PSUM tile.  The host performs the "gather": each core
  outputs its row norms (self-block diagonal), per-row region sums of g and
  g^2 (DVE accum), per-column sums of exp(S2E*g + bias_i) (ACT exp with local
  per-row bias c_i, reduced over rows by a ones-lhsT matmul), and per-column
  sums of g (gpsimd partition_all_reduce).  The host applies the per-column
  factors f_j = exp(S2E*c_j) and all linear c corrections in fp64, with
  c = -(sq - D)/2 so that u = d2 - 2D = -2*(g + c_i + c_j).

  Self blocks are symmetric: the host subtracts the analytically known
  diagonal and halves; distance-8 blocks (strip col 896:1024) are computed
  twice fleet-wide and weighted 0.5.  Every PSUM matmul stays inside a 2KB
  bank and every PSUM region gets a sacrificial start=True zero matmul up
  front (doubles as HAM warm-up); PSUM is strictly write-then-read.
"""

import os

import numpy as np
import ml_dtypes

import concourse.bass as bass
import concourse.bass_isa as bass_isa
import concourse.mybir as mybir
import concourse.tile as tile
from concourse import bacc
from concourse.bass_utils import run_bass_kernel_spmd

NC_N = 8
B, D = 2048, 12288
BLK = 128
KCH = 96
TAU = 0.5
CC = float(2 * D)
S_EXP = 1.0 / (D * TAU)
S2E = 2.0 * S_EXP
F32 = mybir.dt.float32
BF16 = mybir.dt.bfloat16
DT_IN = mybir.dt.float8e4
NP_IN = ml_dtypes.float8_e4m3

N_PAIRS = B * (B - 1) // 2

KERNEL_EXEC_NS = []

_cache = {}

MULT = mybir.AluOpType.mult
ADD = mybir.AluOpType.add
EXP = mybir.ActivationFunctionType.Exp

SEGS = [(0, 512), (512, 1024)]  # strip psum segments (bank-aligned)
REGS = [(0, 896), (896, 1024)]  # strip stats regions (full / half)


def _trace_enabled():
    return bool(os.environ.get("KERNEL_TRACE"))


def _build_kernel():
    nc = bacc.Bacc("TRN2", target_bir_lowering=False, debug=False, num_devices=NC_N)
    xT = nc.dram_tensor("xT", [BLK, KCH, 1280], DT_IN, kind="ExternalInput")
    ident = nc.dram_tensor("ident", [BLK, BLK], F32, kind="ExternalInput")
    zin = nc.dram_tensor("zin", [BLK, 2, 512], DT_IN, kind="ExternalInput")
    onesw = nc.dram_tensor("onesw", [BLK, 4], BF16, kind="ExternalInput")
    out_acc = nc.dram_tensor("out_acc", [BLK, 18], F32, kind="ExternalOutput")
    out_cols = nc.dram_tensor("out_cols", [4, 1280], F32, kind="ExternalOutput")

    DR = mybir.MatmulPerfMode.DoubleRow
    GSIZES = [2, 6] + [8] * 11  # chunks per DMA group (small head for fast start)
    GOFF = [sum(GSIZES[:i]) for i in range(len(GSIZES))]

    with tile.TileContext(nc) as tc:
        with (
            tc.tile_pool(name="slab", bufs=6) as slab_pool,
            tc.tile_pool(name="ps", bufs=1, space="PSUM") as psp,
            tc.tile_pool(name="post", bufs=2) as post,
            tc.tile_pool(name="keep", bufs=1) as keep,
        ):
            ps0 = psp.tile([BLK, 1024], F32, tag="ps0")
            ps1 = psp.tile([BLK, 1024], F32, tag="ps1")
            pd = psp.tile([BLK, 256], F32, tag="pd")
            pcs = psp.tile([2, 1152], F32, tag="pcs")
            ps = [ps0, ps1]
            acc = keep.tile([BLK, 18], F32)

            # --- consts (tiny ow first to absorb DMA-ring warmup, then z2) ---
            ow = keep.tile([BLK, 4], BF16, tag="ow")
            nc.sync.dma_start(ow[:], onesw[:])
            z2 = keep.tile([BLK, 2, 512], DT_IN, tag="z2")
            nc.sync.dma_start(z2[:], zin[:])
            ident_t = keep.tile([BLK, BLK], F32, tag="ident")

            # --- sacrificial start=True zero matmuls (also HAM warm-up) ---
            for s in range(2):
                for a, b in SEGS:
                    nc.tensor.matmul(
                        ps[s][:, a:b], z2[:, 0, 0:128], z2[:, 0, 0 : b - a],
                        start=True, stop=False,
                    )
            for s in range(2):
                nc.tensor.matmul(
                    pd[:, 128 * s : 128 * s + 128], z2[:, 0, 0:128],
                    z2[:, 0, 0:128], start=True, stop=False,
                )
            for a, b in [(0, 512), (512, 1024), (1024, 1152)]:
                nc.tensor.matmul(
                    pcs[:, a:b], z2[:, 0, 0:2], z2[:, 0, 0 : b - a],
                    start=True, stop=False,
                )
            zscr = keep.tile([BLK, 1], F32, tag="zscr")
            nc.scalar.activation(zscr[:], z2[:, 0, 0:1], EXP)

            # --- slab DMAs ---
            xb_tiles = []
            for g, gc in enumerate(GSIZES):
                t = slab_pool.tile([BLK, gc, 1280], DT_IN, tag="slab", name=f"sl{g}")
                nc.sync.dma_start(t[:], xT[:, GOFF[g] : GOFF[g] + gc, :])
                xb_tiles.append(t)
                if g == 6:
                    nc.sync.dma_start(ident_t[:], ident[:])

            def kpair_mms(st, ii, s, last=False):
                lhs = st[:, ii : ii + 2, 128 * s : 128 * s + 128]
                for a, b in SEGS:
                    off = 128 * (s + 1)
                    nc.tensor.matmul(
                        ps[s][:, a:b], lhs,
                        st[:, ii : ii + 2, off + a : off + b],
                        start=False, stop=last, perf_mode=DR,
                    )
                nc.tensor.matmul(
                    pd[:, 128 * s : 128 * s + 128], lhs, lhs,
                    start=False, stop=last, perf_mode=DR,
                )

            # --- Gram accumulation (all but final group) ---
            for g in range(len(GSIZES) - 1):
                st = xb_tiles[g]
                for ii in range(0, GSIZES[g], 2):
                    for s in range(2):
                        kpair_mms(st, ii, s)

            # --- final group: pd first, then seg-major per strip ---
            GC = GSIZES[-1]
            st = xb_tiles[-1]

            def flhs(ii, s):
                return st[:, ii : ii + 2, 128 * s : 128 * s + 128]

            for s in range(2):
                for ii in range(0, GC, 2):
                    nc.tensor.matmul(
                        pd[:, 128 * s : 128 * s + 128], flhs(ii, s), flhs(ii, s),
                        start=False, stop=(ii == GC - 2), perf_mode=DR,
                    )
            etd = keep.tile([BLK, 256], BF16, tag="etd")
            bd = keep.tile([BLK, 256], BF16, tag="bd")
            biases = []
            for s in range(2):
                sl = slice(128 * s, 128 * s + 128)
                escr = post.tile([BLK, BLK], F32, tag="escr")
                nc.vector.scalar_tensor_tensor(
                    out=escr[:], in0=pd[:, sl], scalar=1.0,
                    in1=ident_t[:], op0=MULT, op1=MULT,
                    accum_out=acc[:, 16 + s : 17 + s],
                )
                bias_s = keep.tile([BLK, 1], F32, tag=f"bias{s}", name=f"bias{s}")
                nc.vector.tensor_scalar(
                    out=bias_s[:], in0=acc[:, 16 + s : 17 + s],
                    scalar1=-float(D), scalar2=-0.5 * S2E, op0=ADD, op1=MULT,
                )
                biases.append(bias_s)
                nc.scalar.activation(etd[:, sl], pd[:, sl], EXP, scale=S2E,
                                     bias=bias_s[:])
                nc.vector.tensor_scalar(
                    out=bd[:, sl], in0=pd[:, sl], scalar1=1.0, scalar2=0.0,
                    op0=MULT, op1=ADD, accum_out=acc[:, 12 + s : 13 + s],
                )
                scrd = post.tile([BLK, BLK], F32, tag="scrd")
                nc.vector.scalar_tensor_tensor(
                    out=scrd[:], in0=bd[:, sl], scalar=1.0, in1=pd[:, sl],
                    op0=MULT, op1=MULT, accum_out=acc[:, 14 + s : 15 + s],
                )

            # per strip, seg-major: matmuls then ACT/DVE for that seg
            SUBR = [(0, 512), (512, 896), (896, 1024)]  # accum sub-regions
            et = []
            bt = []
            for s in range(2):
                et_s = keep.tile([BLK, 1024], BF16, tag=f"et{s}", name=f"et{s}")
                b_s = keep.tile([BLK, 1024], BF16, tag=f"b{s}", name=f"b{s}")
                et.append(et_s)
                bt.append(b_s)
                off = 128 * (s + 1)
                for gi, (a, b) in enumerate(SEGS):
                    for ii in range(0, GC, 2):
                        nc.tensor.matmul(
                            ps[s][:, a:b], flhs(ii, s),
                            st[:, ii : ii + 2, off + a : off + b],
                            start=False, stop=(ii == GC - 2), perf_mode=DR,
                        )
                    nc.scalar.activation(
                        et_s[:, a:b], ps[s][:, a:b], EXP, scale=S2E,
                        bias=biases[s][:],
                    )
                    for ra, rb in SUBR:
                        if ra < a or rb > b:
                            continue
                        ri = SUBR.index((ra, rb))
                        nc.vector.tensor_scalar(
                            out=b_s[:, ra:rb], in0=ps[s][:, ra:rb], scalar1=1.0,
                            scalar2=0.0, op0=MULT, op1=ADD,
                            accum_out=acc[:, 3 * s + ri : 3 * s + ri + 1],
                        )
                        scr = post.tile([BLK, rb - ra], F32, tag=f"scr{ri}")
                        nc.vector.scalar_tensor_tensor(
                            out=scr[:], in0=b_s[:, ra:rb], scalar=1.0,
                            in1=ps[s][:, ra:rb], op0=MULT, op1=MULT,
                            accum_out=acc[:, 6 + 3 * s + ri : 7 + 3 * s + ri],
                        )

            nc.sync.dma_start(out_acc[:], acc[:])

            # --- column sums: et -> pcs; g -> overlays on freed ps/pd banks ---
            for s in range(2):
                for a, b in SEGS:
                    nc.tensor.matmul(
                        pcs[:, a:b], ow[:, 2 * s : 2 * s + 2], et[s][:, a:b],
                        start=False, stop=False,
                    )
                nc.tensor.matmul(
                    pcs[:, 1024:1152], ow[:, 2 * s : 2 * s + 2],
                    etd[:, 128 * s : 128 * s + 128],
                    start=False, stop=(s == 1),
                )
                for a, b in SEGS:
                    nc.tensor.matmul(
                        ps[s][0:2, a:b], ow[:, 2 * s : 2 * s + 2], bt[s][:, a:b],
                        start=True, stop=True, skip_group_check=True,
                    )
                nc.tensor.matmul(
                    pd[0:2, 128 * s : 128 * s + 128], ow[:, 2 * s : 2 * s + 2],
                    bd[:, 128 * s : 128 * s + 128],
                    start=True, stop=True, skip_group_check=True,
                )
            cse = keep.tile([2, 1152], F32, tag="cse")
            nc.vector.tensor_copy(cse[:], pcs[:])
            csg = keep.tile([2, 1280], F32, tag="csg")
            nc.vector.tensor_copy(csg[0:2, 0:1024], ps0[0:2, :])
            nc.vector.tensor_tensor(
                out=csg[0:2, 0:1024], in0=csg[0:2, 0:1024], in1=ps1[0:2, :],
                op=mybir.AluOpType.add,
            )
            nc.vector.tensor_copy(csg[0:2, 1024:1152], pd[0:2, 0:128])
            nc.vector.tensor_copy(csg[0:2, 1152:1280], pd[0:2, 128:256])

            nc.sync.dma_start(out_cols[0:2, 0:1152], cse[:])
            nc.sync.dma_start(out_cols[2:4, :], csg[:])
    nc.compile()
    return nc


def _get(name, builder):
    if name not in _cache:
        _cache[name] = builder()
    return _cache[name]


def _run(nc, in_maps, tag):
    if _trace_enabled():
        try:
            import profhook

            profhook.install()
        except Exception:
            pass
        import tempfile

        res = run_bass_kernel_spmd(
            nc, in_maps, list(range(NC_N)), trace=True,
            tmpdir=tempfile.mkdtemp(prefix=f"ktrace_{tag}_"),
        )
        KERNEL_EXEC_NS.append((tag, res.exec_time_ns))
        return res.results
    return run_bass_kernel_spmd(nc, in_maps, list(range(NC_N))).results


def kernel(features):
    x = np.asarray(features).reshape(B, D)
    xbf = x.astype(NP_IN)
    xT_full = np.ascontiguousarray(xbf.T)  # (D, B)

    ident = np.eye(BLK, dtype=np.float32)
    zin = np.zeros((BLK, 2, 512), dtype=NP_IN)
    onesw = np.zeros((BLK, 4), dtype=ml_dtypes.bfloat16)
    onesw[:, 0] = 1.0  # strip0 lhsT -> psum row 0
    onesw[:, 3] = 1.0  # strip1 lhsT -> psum row 1
    in_maps = []
    for c in range(NC_N):
        cols = (256 * c + np.arange(1280)) % B
        xu = xT_full[:, cols].reshape(KCH, BLK, 1280).transpose(1, 0, 2)
        in_maps.append(
            {
                "xT": np.ascontiguousarray(xu),
                "ident": ident,
                "zin": zin,
                "onesw": onesw,
            }
        )
    nc = _get("main", _build_kernel)
    res = _run(nc, in_maps, "main")

    # ---- host combine (fp64) ----
    sq = np.zeros(B)
    for c in range(NC_N):
        a = res[c]["out_acc"].astype(np.float64)
        sq[256 * c : 256 * c + 128] = a[:, 16]
        sq[256 * c + 128 : 256 * c + 256] = a[:, 17]
    cvec = -(sq - D) / 2.0
    s2e = np.float64(np.float32(S2E))
    fvec = np.exp(s2e * cvec)
    sq_b16 = sq.astype(np.float32).astype(ml_dtypes.bfloat16).astype(np.float64)

    E = S1 = S2 = 0.0
    for c in range(NC_N):
        A = res[c]["out_acc"].astype(np.float64)
        CSraw = res[c]["out_cols"].astype(np.float64)
        CS = np.zeros((4, 1152))
        CS[0:2, :] = CSraw[0:2, 0:1152]
        CS[2:4, 0:1024] = CSraw[2:4, 0:1024]
        CS[2, 1024:1152] = CSraw[2, 1024:1152]
        CS[3, 1024:1152] = CSraw[3, 1152:1280]
        for s in range(2):
            rows = 256 * c + 128 * s + np.arange(128)
            scols = (256 * c + 128 * (s + 1) + np.arange(1024)) % B
            c_row = cvec[rows]
            # strip regions: full (w=1, sub-cols 0+1), half (w=0.5, sub-col 2)
            for r, (a, b) in enumerate(REGS):
                W = b - a
                cR = cvec[scols[a:b]]
                if r == 0:
                    s1rows = A[:, 3 * s] + A[:, 3 * s + 1]
                    S2g = A[:, 6 + 3 * s].sum() + A[:, 7 + 3 * s].sum()
                else:
                    s1rows = A[:, 3 * s + 2]
                    S2g = A[:, 8 + 3 * s].sum()
                S1g = s1rows.sum()
                E_reg = (fvec[scols[a:b]] * CS[s, a:b]).sum()
                S1_reg = S1g + W * c_row.sum() + 128.0 * cR.sum()
                S2_reg = (
                    S2g
                    + 2.0 * (c_row * s1rows).sum()
                    + 2.0 * (cR * CS[2 + s, a:b]).sum()
                    + W * (c_row**2).sum()
                    + 2.0 * c_row.sum() * cR.sum()
                    + 128.0 * (cR**2).sum()
                )
                w = 1.0 if r == 0 else 0.5
                E += w * E_reg
                S1 += w * S1_reg
                S2 += w * S2_reg
            # self block: remove diagonal, halve
            cR = c_row
            s1rows = A[:, 12 + s]
            S1g = s1rows.sum()
            S2g = A[:, 14 + s].sum()
            E_reg = (fvec[rows] * CS[s, 1024:1152]).sum()
            S1_reg = S1g + 128.0 * c_row.sum() + 128.0 * cR.sum()
            S2_reg = (
                S2g
                + 2.0 * (c_row * s1rows).sum()
                + 2.0 * (cR * CS[2 + s, 1024:1152]).sum()
                + 128.0 * (c_row**2).sum()
                + 2.0 * c_row.sum() * cR.sum()
                + 128.0 * (cR**2).sum()
            )
            sqr = sq[rows]
            bqr = sq_b16[rows]
            E_dd = (fvec[rows] * np.exp(s2e * (sqr + c_row))).sum()
            S1_dd = (bqr + 2.0 * c_row).sum()
            S2_dd = (bqr * sqr + 4.0 * c_row * bqr + 4.0 * c_row**2).sum()
            E += 0.5 * (E_reg - E_dd)
            S1 += 0.5 * (S1_reg - S1_dd)
            S2 += 0.5 * (S2_reg - S2_dd)

    sum_u = -2.0 * S1
    sum_u2 = 4.0 * S2
    N = float(N_PAIRS)
    mean_u = sum_u / N
    mean = (mean_u + CC) / D
    var_u = (sum_u2 - N * mean_u * mean_u) / (N - 1.0)
    std = np.sqrt(var_u) / D
    loss = CC * S_EXP - np.log(E) + np.log(N)
    feat_norm = np.sqrt(sq).mean()

    return (
        np.float32(loss),
        np.float32(feat_norm),
        np.float32(mean),
        np.float32(std),
    )


if __name__ == "__main__":
    f = np.random.default_rng(0).standard_normal((B, 16, 768), dtype=np.float32)
    print(kernel(features=f))


# revision 32
# speedup vs baseline: 1.0258x; 1.0258x over previous
"""Trainium2 Bass kernel for nn_DispersiveLoss (B=2048, D=16*768=12288, 8 cores).

Single-launch circulant scheme, no cross-core communication:
  x (2048, 12288) -> 16 row-blocks of 128. Core c owns m-blocks {2c, 2c+1} and
  computes two raw Gram strips G[m, m+1..m+8 (mod 16)] ([128,1024], fp8
  DoubleRow, D on partitions, fp32 PSUM) plus the two self blocks G[m,m] in a
  separate [128,256] PSUM tile.  The host performs the "gather": each core
  outputs its row norms (self-block diagonal), per-row region sums of g and
  g^2 (DVE accum), per-column sums of exp(S2E*g + bias_i) (ACT exp with local
  per-row bias c_i, reduced over rows by a ones-lhsT matmul), and per-column
  sums of g (ones-lhsT matmuls into freed PSUM banks).  The host applies the
  factors f_j = exp(S2E*c_j) and all linear c corrections in fp64, with
  c = -(sq - D)/2 so that u = d2 - 2D = -2*(g + c_i + c_j).

  Self blocks are symmetric: the host subtracts the analytically known
  diagonal and halves; distance-8 blocks (strip col 896:1024) are computed
  twice fleet-wide and weighted 0.5.  Every PSUM matmul stays inside a 2KB
  bank and every PSUM region gets a sacrificial start=True zero matmul up
  front (doubles as HAM warm-up); PSUM is strictly write-then-read.
"""

import os

import numpy as np
import ml_dtypes

import concourse.bass as bass
import concourse.mybir as mybir
import concourse.tile as tile
from concourse import bacc
from concourse.bass_utils import run_bass_kernel_spmd

NC_N = 8
B, D = 2048, 12288
BLK = 128
KCH = 96
TAU = 0.5
CC = float(2 * D)
S_EXP = 1.0 / (D * TAU)
S2E = 2.0 * S_EXP
F32 = mybir.dt.float32
BF16 = mybir.dt.bfloat16
DT_IN = mybir.dt.float8e4
NP_IN = ml_dtypes.float8_e4m3

N_PAIRS = B * (B - 1) // 2

KERNEL_EXEC_NS = []

_cache = {}

MULT = mybir.AluOpType.mult
ADD = mybir.AluOpType.add
EXP = mybir.ActivationFunctionType.Exp

SEGS = [(0, 512), (512, 1024)]  # strip psum segments (bank-aligned)
REGS = [(0, 896), (896, 1024)]  # strip stats regions (full / half)


def _trace_enabled():
    return bool(os.environ.get("KERNEL_TRACE"))


def _build_kernel():
    nc = bacc.Bacc("TRN2", target_bir_lowering=False, debug=False, num_devices=NC_N)
    xT = nc.dram_tensor("xT", [BLK, KCH, 1280], DT_IN, kind="ExternalInput")
    ident = nc.dram_tensor("ident", [BLK, BLK], F32, kind="ExternalInput")
    zin = nc.dram_tensor("zin", [BLK, 2, 512], DT_IN, kind="ExternalInput")
    onesw = nc.dram_tensor("onesw", [BLK, 4], BF16, kind="ExternalInput")
    out_acc = nc.dram_tensor("out_acc", [BLK, 18], F32, kind="ExternalOutput")
    out_cols = nc.dram_tensor("out_cols", [4, 1280], F32, kind="ExternalOutput")

    DR = mybir.MatmulPerfMode.DoubleRow
    GSIZES = [2, 6] + [8] * 11  # chunks per DMA group (small head for fast start)
    GOFF = [sum(GSIZES[:i]) for i in range(len(GSIZES))]

    with tile.TileContext(nc) as tc:
        with (
            tc.tile_pool(name="slab", bufs=6) as slab_pool,
            tc.tile_pool(name="ps", bufs=1, space="PSUM") as psp,
            tc.tile_pool(name="post", bufs=2) as post,
            tc.tile_pool(name="keep", bufs=1) as keep,
        ):
            ps0 = psp.tile([BLK, 1024], F32, tag="ps0")
            ps1 = psp.tile([BLK, 1024], F32, tag="ps1")
            pd = psp.tile([BLK, 256], F32, tag="pd")
            pcs = psp.tile([2, 1152], F32, tag="pcs")
            ps = [ps0, ps1]
            acc = keep.tile([BLK, 18], F32)

            # --- consts (tiny ow first to absorb DMA-ring warmup, then z2) ---
            ow = keep.tile([BLK, 4], BF16, tag="ow")
            nc.sync.dma_start(ow[:], onesw[:])
            z2 = keep.tile([BLK, 2, 512], DT_IN, tag="z2")
            nc.sync.dma_start(z2[:], zin[:])
            ident_t = keep.tile([BLK, BLK], F32, tag="ident")

            # --- sacrificial start=True zero matmuls (also HAM warm-up) ---
            for s in range(2):
                for a, b in SEGS:
                    nc.tensor.matmul(
                        ps[s][:, a:b], z2[:, 0, 0:128], z2[:, 0, 0 : b - a],
                        start=True, stop=False,
                    )
            for s in range(2):
                nc.tensor.matmul(
                    pd[:, 128 * s : 128 * s + 128], z2[:, 0, 0:128],
                    z2[:, 0, 0:128], start=True, stop=False,
                )
            for a, b in [(0, 512), (512, 1024), (1024, 1152)]:
                nc.tensor.matmul(
                    pcs[:, a:b], z2[:, 0, 0:2], z2[:, 0, 0 : b - a],
                    start=True, stop=False,
                )
            zscr = keep.tile([BLK, 1], F32, tag="zscr")
            nc.scalar.activation(zscr[:], z2[:, 0, 0:1], EXP)

            # --- slab DMAs ---
            xb_tiles = []
            for g, gc in enumerate(GSIZES):
                t = slab_pool.tile([BLK, gc, 1280], DT_IN, tag="slab", name=f"sl{g}")
                nc.sync.dma_start(t[:], xT[:, GOFF[g] : GOFF[g] + gc, :])
                xb_tiles.append(t)
                if g == 6:
                    nc.sync.dma_start(ident_t[:], ident[:])

            def kpair_mms(st, ii, s, last=False):
                lhs = st[:, ii : ii + 2, 128 * s : 128 * s + 128]
                for a, b in SEGS:
                    off = 128 * (s + 1)
                    nc.tensor.matmul(
                        ps[s][:, a:b], lhs,
                        st[:, ii : ii + 2, off + a : off + b],
                        start=False, stop=last, perf_mode=DR,
                    )
                nc.tensor.matmul(
                    pd[:, 128 * s : 128 * s + 128], lhs, lhs,
                    start=False, stop=last, perf_mode=DR,
                )

            # --- Gram accumulation (all but final group) ---
            for g in range(len(GSIZES) - 1):
                st = xb_tiles[g]
                for ii in range(0, GSIZES[g], 2):
                    for s in range(2):
                        kpair_mms(st, ii, s)

            # --- final group: pd first, then seg-major per strip ---
            GC = GSIZES[-1]
            st = xb_tiles[-1]

            def flhs(ii, s):
                return st[:, ii : ii + 2, 128 * s : 128 * s + 128]

            for s in range(2):
                for ii in range(0, GC, 2):
                    nc.tensor.matmul(
                        pd[:, 128 * s : 128 * s + 128], flhs(ii, s), flhs(ii, s),
                        start=False, stop=(ii == GC - 2), perf_mode=DR,
                    )
            etd = keep.tile([BLK, 256], BF16, tag="etd")
            bd = keep.tile([BLK, 256], BF16, tag="bd")
            biases = []
            for s in range(2):
                sl = slice(128 * s, 128 * s + 128)
                escr = post.tile([BLK, BLK], F32, tag="escr")
                nc.vector.scalar_tensor_tensor(
                    out=escr[:], in0=pd[:, sl], scalar=1.0,
                    in1=ident_t[:], op0=MULT, op1=MULT,
                    accum_out=acc[:, 16 + s : 17 + s],
                )
                bias_s = keep.tile([BLK, 1], F32, tag=f"bias{s}", name=f"bias{s}")
                nc.vector.tensor_scalar(
                    out=bias_s[:], in0=acc[:, 16 + s : 17 + s],
                    scalar1=-float(D), scalar2=-0.5 * S2E, op0=ADD, op1=MULT,
                )
                biases.append(bias_s)
                nc.scalar.activation(etd[:, sl], pd[:, sl], EXP, scale=S2E,
                                     bias=bias_s[:])
                nc.vector.tensor_scalar(
                    out=bd[:, sl], in0=pd[:, sl], scalar1=1.0, scalar2=0.0,
                    op0=MULT, op1=ADD, accum_out=acc[:, 12 + s : 13 + s],
                )
                scrd = post.tile([BLK, BLK], F32, tag="scrd")
                nc.vector.scalar_tensor_tensor(
                    out=scrd[:], in0=bd[:, sl], scalar=1.0, in1=pd[:, sl],
                    op0=MULT, op1=MULT, accum_out=acc[:, 14 + s : 15 + s],
                )

            # per strip, seg-major: matmuls then ACT/DVE for that seg
            SUBR = [(0, 512), (512, 896), (896, 1024)]  # accum sub-regions
            et = []
            bt = []
            for s in range(2):
                et_s = keep.tile([BLK, 1024], BF16, tag=f"et{s}", name=f"et{s}")
                b_s = keep.tile([BLK, 1024], BF16, tag=f"b{s}", name=f"b{s}")
                et.append(et_s)
                bt.append(b_s)
                off = 128 * (s + 1)
                for gi, (a, b) in enumerate(SEGS):
                    for ii in range(0, GC, 2):
                        nc.tensor.matmul(
                            ps[s][:, a:b], flhs(ii, s),
                            st[:, ii : ii + 2, off + a : off + b],
                            start=False, stop=(ii == GC - 2), perf_mode=DR,
                        )
                    nc.scalar.activation(
                        et_s[:, a:b], ps[s][:, a:b], EXP, scale=S2E,
                        bias=biases[s][:],
                    )
                    for ra, rb in SUBR:
                        if ra < a or rb > b:
                            continue
                        ri = SUBR.index((ra, rb))
                        nc.vector.tensor_scalar(
                            out=b_s[:, ra:rb], in0=ps[s][:, ra:rb], scalar1=1.0,
                            scalar2=0.0, op0=MULT, op1=ADD,
                            accum_out=acc[:, 3 * s + ri : 3 * s + ri + 1],
                        )
                        scr = post.tile([BLK, rb - ra], F32, tag=f"scr{ri}")
                        nc.vector.scalar_tensor_tensor(
                            out=scr[:], in0=b_s[:, ra:rb], scalar=1.0,
                            in1=ps[s][:, ra:rb], op0=MULT, op1=MULT,
                            accum_out=acc[:, 6 + 3 * s + ri : 7 + 3 * s + ri],
                        )

            nc.sync.dma_start(out_acc[:], acc[:])

            # --- column sums: et -> pcs; g -> overlays on freed ps/pd banks ---
            for s in range(2):
                for a, b in SEGS:
                    nc.tensor.matmul(
                        pcs[:, a:b], ow[:, 2 * s : 2 * s + 2], et[s][:, a:b],
                        start=False, stop=False,
                    )
                nc.tensor.matmul(
                    pcs[:, 1024:1152], ow[:, 2 * s : 2 * s + 2],
                    etd[:, 128 * s : 128 * s + 128],
                    start=False, stop=(s == 1),
                )
                for a, b in SEGS:
                    nc.tensor.matmul(
                        ps[s][0:2, a:b], ow[:, 2 * s : 2 * s + 2], bt[s][:, a:b],
                        start=True, stop=True, skip_group_check=True,
                    )
                nc.tensor.matmul(
                    pd[0:2, 128 * s : 128 * s + 128], ow[:, 2 * s : 2 * s + 2],
                    bd[:, 128 * s : 128 * s + 128],
                    start=True, stop=True, skip_group_check=True,
                )
            cse = keep.tile([2, 1152], F32, tag="cse")
            nc.vector.tensor_copy(cse[:], pcs[:])
            csg = keep.tile([2, 1280], F32, tag="csg")
            nc.vector.tensor_copy(csg[0:2, 0:1024], ps0[0:2, :])
            nc.vector.tensor_tensor(
                out=csg[0:2, 0:1024], in0=csg[0:2, 0:1024], in1=ps1[0:2, :],
                op=mybir.AluOpType.add,
            )
            nc.vector.tensor_copy(csg[0:2, 1024:1152], pd[0:2, 0:128])
            nc.vector.tensor_copy(csg[0:2, 1152:1280], pd[0:2, 128:256])

            nc.sync.dma_start(out_cols[0:2, 0:1152], cse[:])
            nc.sync.dma_start(out_cols[2:4, :], csg[:])
    nc.compile()
    return nc


def _get(name, builder):
    if name not in _cache:
        _cache[name] = builder()
    return _cache[name]


def _run(nc, in_maps, tag):
    if _trace_enabled():
        try:
            import profhook

            profhook.install()
        except Exception:
            pass
        import tempfile

        res = run_bass_kernel_spmd(
            nc, in_maps, list(range(NC_N)), trace=True,
            tmpdir=tempfile.mkdtemp(prefix=f"ktrace_{tag}_"),
        )
        KERNEL_EXEC_NS.append((tag, res.exec_time_ns))
        return res.results
    return run_bass_kernel_spmd(nc, in_maps, list(range(NC_N))).results


def kernel(features):
    x = np.asarray(features).reshape(B, D)
    xbf = x.astype(NP_IN)
    xT_full = np.ascontiguousarray(xbf.T)  # (D, B)

    ident = np.eye(BLK, dtype=np.float32)
    zin = np.zeros((BLK, 2, 512), dtype=NP_IN)
    onesw = np.zeros((BLK, 4), dtype=ml_dtypes.bfloat16)
    onesw[:, 0] = 1.0  # strip0 lhsT -> psum row 0
    onesw[:, 3] = 1.0  # strip1 lhsT -> psum row 1
    in_maps = []
    for c in range(NC_N):
        cols = (256 * c + np.arange(1280)) % B
        xu = xT_full[:, cols].reshape(KCH, BLK, 1280).transpose(1, 0, 2)
        in_maps.append(
            {
                "xT": np.ascontiguousarray(xu),
                "ident": ident,
                "zin": zin,
                "onesw": onesw,
            }
        )
    nc = _get("main", _build_kernel)
    res = _run(nc, in_maps, "main")

    # ---- host combine (fp64) ----
    sq = np.zeros(B)
    for c in range(NC_N):
        a = res[c]["out_acc"].astype(np.float64)
        sq[256 * c : 256 * c + 128] = a[:, 16]
        sq[256 * c + 128 : 256 * c + 256] = a[:, 17]
    cvec = -(sq - D) / 2.0
    s2e = np.float64(np.float32(S2E))
    fvec = np.exp(s2e * cvec)
    sq_b16 = sq.astype(np.float32).astype(ml_dtypes.bfloat16).astype(np.float64)

    E = S1 = S2 = 0.0
    for c in range(NC_N):
        A = res[c]["out_acc"].astype(np.float64)
        CSraw = res[c]["out_cols"].astype(np.float64)
        CS = np.zeros((4, 1152))
        CS[0:2, :] = CSraw[0:2, 0:1152]
        CS[2:4, 0:1024] = CSraw[2:4, 0:1024]
        CS[2, 1024:1152] = CSraw[2, 1024:1152]
        CS[3, 1024:1152] = CSraw[3, 1152:1280]
        for s in range(2):
            rows = 256 * c + 128 * s + np.arange(128)
            scols = (256 * c + 128 * (s + 1) + np.arange(1024)) % B
            c_row = cvec[rows]
            # strip regions: full (w=1, sub-cols 0+1), half (w=0.5, sub-col 2)
            for r, (a, b) in enumerate(REGS):
                W = b - a
                cR = cvec[scols[a:b]]
                if r == 0:
                    s1rows = A[:, 3 * s] + A[:, 3 * s + 1]
                    S2g = A[:, 6 + 3 * s].sum() + A[:, 7 + 3 * s].sum()
                else:
                    s1rows = A[:, 3 * s + 2]
                    S2g = A[:, 8 + 3 * s].sum()
                S1g = s1rows.sum()
                E_reg = (fvec[scols[a:b]] * CS[s, a:b]).sum()
                S1_reg = S1g + W * c_row.sum() + 128.0 * cR.sum()
                S2_reg = (
                    S2g
                    + 2.0 * (c_row * s1rows).sum()
                    + 2.0 * (cR * CS[2 + s, a:b]).sum()
                    + W * (c_row**2).sum()
                    + 2.0 * c_row.sum() * cR.sum()
                    + 128.0 * (cR**2).sum()
                )
                w = 1.0 if r == 0 else 0.5
                E += w * E_reg
                S1 += w * S1_reg
                S2 += w * S2_reg
            # self block: remove diagonal, halve
            cR = c_row
            s1rows = A[:, 12 + s]
            S1g = s1rows.sum()
            S2g = A[:, 14 + s].sum()
            E_reg = (fvec[rows] * CS[s, 1024:1152]).sum()
            S1_reg = S1g + 128.0 * c_row.sum() + 128.0 * cR.sum()
            S2_reg = (
                S2g
                + 2.0 * (c_row * s1rows).sum()
                + 2.0 * (cR * CS[2 + s, 1024:1152]).sum()
                + 128.0 * (c_row**2).sum()
                + 2.0 * c_row.sum() * cR.sum()
                + 128.0 * (cR**2).sum()
            )
            sqr = sq[rows]
            bqr = sq_b16[rows]
            E_dd = (fvec[rows] * np.exp(s2e * (sqr + c_row))).sum()
            S1_dd = (bqr + 2.0 * c_row).sum()
            S2_dd = (bqr * sqr + 4.0 * c_row * bqr + 4.0 * c_row**2).sum()
            E += 0.5 * (E_reg - E_dd)
            S1 += 0.5 * (S1_reg - S1_dd)
            S2 += 0.5 * (S2_reg - S2_dd)

    sum_u = -2.0 * S1
    sum_u2 = 4.0 * S2
    N = float(N_PAIRS)
    mean_u = sum_u / N
    mean = (mean_u + CC) / D
    var_u = (sum_u2 - N * mean_u * mean_u) / (N - 1.0)
    std = np.sqrt(var_u) / D
    loss = CC * S_EXP - np.log(E) + np.log(N)
    feat_norm = np.sqrt(sq).mean()

    return (
        np.float32(loss),
        np.float32(feat_norm),
        np.float32(mean),
        np.float32(std),
    )


if __name__ == "__main__":
    f = np.random.default_rng(0).standard_normal((B, 16, 768), dtype=np.float32)
    print(kernel(features=f))
